# revision 4
# baseline (speedup 1.0000x reference)
"""CfCActorCritic forward_sequence kernel for Trainium2 (Bass/Tile), 8 cores.

Math (reference):
  per step t:  z  = 1.7159*tanh(0.666*([x_t, h] @ Wb + bb))
               ff1 = tanh(z @ Wff1 + bff1)
               ff2 = tanh(z @ Wff2 + bff2)
               ti  = sigmoid(z @ (Wta+Wtb) + bta+btb)        (ts = 1.0)
               h   = ff1*(1-ti) + ti*ff2
  heads: mean = out @ Wa + ba ; value = out @ Wc + bc  (per timestep)

Kernel reformulation (all algebraic, exact in fp32):
  - store state as h' = 2h; then with s = tanh((z@Wtab' + btab')),
    Wtab' = 0.5*1.7159*(Wta+Wtb), btab' = 0.5*(bta+btb):
      ti = (1+s)/2  and  h' = (1+s)*ff2 + (1-s)*ff1
    computed as p=(s+1)*ff2, q=(s-1)*ff1, h'=p-q  (2 STT + 1 TT on DVE)
  - 1.7159 folded into Wff1/Wff2/Wtab; 0.666 folded into Wb parts and bb;
    0.5 (from h'=2h) folded into Wb_h and the head weights.
  - obs projection x_t @ Wbx is batched: one N=512 matmul per 16-step chunk
    accumulated into a PSUM bank; the recurrent h@Wbh matmul accumulates
    into the same bank slice, so tanh_z reads PSUM directly (ACT bias=bb).
  - actor+critic heads are one [128,9] weight; applied once per chunk on the
    accumulated hidden-state tile [128, 512] (features x (t,b)).

Sharding: batch 256 -> 32 per core (data parallel, 8 cores); weights
replicated; scan over T=1024 sequential per core.
"""

import os
import sys
import types

import numpy as np

import concourse.bass as bass
import concourse.tile as tile
from concourse import mybir
from concourse.bass import ds

# ---------------------------------------------------------------- constants
B, T, OBS, H, A = 256, 1024, 32, 128, 8
NCORES = 8
BL = B // NCORES          # 32 batch per core
TC = 16                   # timesteps per chunk
NCH = T // TC             # 64 chunks
CW = TC * BL              # 512 chunk free width
NHEAD = A + 1             # actor(8) + critic(1)

_F32 = mybir.dt.float32

_last_results = None      # BassKernelResults from the most recent run


def _split_multi_waits(nc):
    """This toolchain's codegen rejects instructions carrying more than one
    sync-wait command. Hoist extra waits onto same-engine NOPs inserted just
    before the instruction (engine blocks on each in order — semantics
    identical, issue order per engine preserved)."""
    n_split = 0
    for f in nc.m.functions:
        for blk in f.blocks:
            new_insts = []
            for inst in blk.instructions:
                si = inst.sync_info
                if si is not None and len(si.on_wait) > 1:
                    waits = list(si.on_wait)
                    for k, w in enumerate(waits[:-1]):
                        nop = mybir.InstNoOp(
                            name=f"{inst.name}_sw{k}",
                            engine=inst.engine,
                            sync_info=mybir.SyncInfo(on_wait=[w], on_update=[]),
                        )
                        new_insts.append(nop)
                        n_split += 1
                    inst.sync_info = mybir.SyncInfo(
                        on_wait=[waits[-1]], on_update=list(si.on_update)
                    )
                new_insts.append(inst)
            blk.instructions[:] = new_insts
    return n_split


def _build_program():
    nc = bass.Bass()
    f32 = _F32

    obs_r = nc.dram_tensor("obs_r", [NCH, OBS, CW], f32, kind="ExternalInput")
    wbx = nc.dram_tensor("wbx", [OBS, H], f32, kind="ExternalInput")
    wbh = nc.dram_tensor("wbh", [H, H], f32, kind="ExternalInput")
    wff1 = nc.dram_tensor("wff1", [H, H], f32, kind="ExternalInput")
    wff2 = nc.dram_tensor("wff2", [H, H], f32, kind="ExternalInput")
    wtab = nc.dram_tensor("wtab", [H, H], f32, kind="ExternalInput")
    wac = nc.dram_tensor("wac", [H, NHEAD], f32, kind="ExternalInput")
    bbe = nc.dram_tensor("bbe", [H, 1], f32, kind="ExternalInput")
    bff1 = nc.dram_tensor("bff1", [H, 1], f32, kind="ExternalInput")
    bff2 = nc.dram_tensor("bff2", [H, 1], f32, kind="ExternalInput")
    btab = nc.dram_tensor("btab", [H, 1], f32, kind="ExternalInput")
    bac = nc.dram_tensor("bac", [NHEAD, 1], f32, kind="ExternalInput")
    out = nc.dram_tensor("headT", [NHEAD, T * BL], f32, kind="ExternalOutput")

    Tanh = mybir.ActivationFunctionType.Tanh
    Identity = mybir.ActivationFunctionType.Identity
    ADD = mybir.AluOpType.add
    SUB = mybir.AluOpType.subtract
    MULT = mybir.AluOpType.mult

    with tile.TileContext(nc) as tc:
        with (
            tc.tile_pool(name="const", bufs=1) as const,
            tc.tile_pool(name="obsp", bufs=3) as obsp,
            tc.tile_pool(name="hchunk", bufs=2) as hchunkp,
            tc.tile_pool(name="outp", bufs=3) as outp,
            tc.tile_pool(name="zp", bufs=3) as zp,
            tc.tile_pool(name="fzp", bufs=3) as fzp,
            tc.tile_pool(name="tmpp", bufs=4) as tmpp,
            tc.tile_pool(name="xpp", bufs=2, space="PSUM") as xpp,
            tc.tile_pool(name="fpp", bufs=3, space="PSUM") as fpp,
            tc.tile_pool(name="hdp", bufs=2, space="PSUM") as hdp,
        ):
            # --- load constants into SBUF ---
            wbx_s = const.tile([OBS, H], f32)
            nc.sync.dma_start(out=wbx_s, in_=wbx[:])
            wbh_s = const.tile([H, H], f32)
            nc.sync.dma_start(out=wbh_s, in_=wbh[:])
            wff1_s = const.tile([H, H], f32)
            nc.sync.dma_start(out=wff1_s, in_=wff1[:])
            wff2_s = const.tile([H, H], f32)
            nc.sync.dma_start(out=wff2_s, in_=wff2[:])
            wtab_s = const.tile([H, H], f32)
            nc.sync.dma_start(out=wtab_s, in_=wtab[:])
            wac_s = const.tile([H, NHEAD], f32)
            nc.sync.dma_start(out=wac_s, in_=wac[:])
            bbe_s = const.tile([H, 1], f32)
            nc.sync.dma_start(out=bbe_s, in_=bbe[:])
            bff1_s = const.tile([H, 1], f32)
            nc.sync.dma_start(out=bff1_s, in_=bff1[:])
            bff2_s = const.tile([H, 1], f32)
            nc.sync.dma_start(out=bff2_s, in_=bff2[:])
            btab_s = const.tile([H, 1], f32)
            nc.sync.dma_start(out=btab_s, in_=btab[:])
            bac_s = const.tile([NHEAD, 1], f32)
            nc.sync.dma_start(out=bac_s, in_=bac[:])

            prev_h = None
            for ch in range(NCH):
                obs_t = obsp.tile([OBS, CW], f32)
                nc.sync.dma_start(out=obs_t, in_=obs_r[ch])
                # xp[:, tc*32:(tc+1)*32] = 0.666 * x_t @ Wbx for the 16 steps
                xp = xpp.tile([H, CW], f32)
                nc.tensor.matmul(
                    xp, wbx_s, obs_t, start=True, stop=False, skip_group_check=True
                )
                hc = hchunkp.tile([H, CW], f32)
                for tcl in range(TC):
                    sl = ds(tcl * BL, BL)
                    if prev_h is not None:
                        nc.tensor.matmul(
                            xp[:, sl],
                            wbh_s,
                            prev_h,
                            start=False,
                            stop=True,
                            skip_group_check=True,
                        )
                    z = zp.tile([H, BL], f32)
                    nc.scalar.activation(z, xp[:, sl], Tanh, bias=bbe_s)

                    f_ps = fpp.tile([H, 3 * BL], f32)
                    nc.tensor.matmul(
                        f_ps[:, ds(0, BL)], wff1_s, z, start=True, stop=True
                    )
                    nc.tensor.matmul(
                        f_ps[:, ds(BL, BL)], wff2_s, z, start=True, stop=True
                    )
                    nc.tensor.matmul(
                        f_ps[:, ds(2 * BL, BL)], wtab_s, z, start=True, stop=True
                    )
                    fz = fzp.tile([H, 3 * BL], f32)
                    nc.scalar.activation(
                        fz[:, ds(0, BL)], f_ps[:, ds(0, BL)], Tanh, bias=bff1_s
                    )
                    nc.scalar.activation(
                        fz[:, ds(BL, BL)], f_ps[:, ds(BL, BL)], Tanh, bias=bff2_s
                    )
                    nc.scalar.activation(
                        fz[:, ds(2 * BL, BL)], f_ps[:, ds(2 * BL, BL)], Tanh,
                        bias=btab_s,
                    )
                    ff1 = fz[:, ds(0, BL)]
                    ff2 = fz[:, ds(BL, BL)]
                    s = fz[:, ds(2 * BL, BL)]
                    # h' = (1+s)*ff2 + (1-s)*ff1  ==  p - q
                    p = tmpp.tile([H, BL], f32)
                    nc.vector.scalar_tensor_tensor(p, s, 1.0, ff2, ADD, MULT)
                    q = tmpp.tile([H, BL], f32)
                    nc.vector.scalar_tensor_tensor(q, s, 1.0, ff1, SUB, MULT)
                    nc.vector.tensor_tensor(hc[:, sl], p, q, SUB)
                    prev_h = hc[:, sl]

                hd = hdp.tile([NHEAD, CW], f32)
                nc.tensor.matmul(hd, wac_s, hc, start=True, stop=True)
                ot = outp.tile([NHEAD, CW], f32)
                nc.scalar.activation(ot, hd, Identity, bias=bac_s)
                nc.sync.dma_start(out=out[:, ds(ch * CW, CW)], in_=ot)

    n = _split_multi_waits(nc)
    return nc


def _prep_inputs(obs, Wb, bb, Wff1, bff1, Wff2, bff2, Wta, bta, Wtb, btb,
                 Wa, ba, Wc, bc):
    """Fold scales into weights; reshape obs per-core. Returns list of in_maps."""
    a = 0.666
    g = 1.7159
    wbx = np.ascontiguousarray(a * Wb[:OBS, :], np.float32)
    wbh = np.ascontiguousarray(0.5 * a * Wb[OBS:, :], np.float32)
    bbe = (a * bb).astype(np.float32).reshape(H, 1)
    wff1 = np.ascontiguousarray(g * Wff1, np.float32)
    wff2 = np.ascontiguousarray(g * Wff2, np.float32)
    wtab = np.ascontiguousarray(0.5 * g * (Wta + Wtb), np.float32)
    btab = (0.5 * (bta + btb)).astype(np.float32).reshape(H, 1)
    wac = np.ascontiguousarray(
        0.5 * np.concatenate([Wa, Wc], axis=1), np.float32
    )  # [128, 9]
    bac = np.concatenate([ba, bc]).astype(np.float32).reshape(NHEAD, 1)

    # obs_r[c, ch, o, tc, b] = obs[c*BL + b, ch*TC + tc, o]
    obs_r = np.ascontiguousarray(
        obs.reshape(NCORES, BL, NCH, TC, OBS).transpose(0, 2, 4, 3, 1),
        np.float32,
    ).reshape(NCORES, NCH, OBS, CW)

    shared = dict(
        wbx=wbx, wbh=wbh, wff1=wff1, wff2=wff2, wtab=wtab, wac=wac,
        bbe=bbe, bff1=bff1.astype(np.float32).reshape(H, 1),
        bff2=bff2.astype(np.float32).reshape(H, 1), btab=btab, bac=bac,
    )
    return [dict(shared, obs_r=obs_r[c]) for c in range(NCORES)]


def _ensure_ntff_hook():
    """Register the axon NTFF profiling hook if the image's antenv lacks it."""
    try:
        from antenv.axon_hooks import get_axon_ntff_profile_hook  # noqa: F401
        return
    except ImportError:
        pass
    import antenv

    mod = types.ModuleType("antenv.axon_hooks")
    _h = [None]
    mod.set_axon_ntff_profile_hook = lambda hook: _h.__setitem__(0, hook)
    mod.get_axon_ntff_profile_hook = lambda: _h[0]
    sys.modules["antenv.axon_hooks"] = mod
    antenv.axon_hooks = mod
    try:
        from trn_agent_boot.trn_boot import _ntff_profile_via_ctypes

        mod.set_axon_ntff_profile_hook(
            _ntff_profile_via_ctypes("/opt/axon/libaxon_pjrt.so")
        )
    except Exception:
        pass


def kernel(obs, Wb, bb, Wff1, bff1, Wff2, bff2, Wta, bta, Wtb, btb,
           Wa, ba, Wc, bc):
    global _last_results
    from concourse.bass_utils import run_bass_kernel_spmd

    trace = os.environ.get("CFC_TRACE", "") == "1"
    if trace:
        _ensure_ntff_hook()

    in_maps = _prep_inputs(obs, Wb, bb, Wff1, bff1, Wff2, bff2, Wta, bta,
                           Wtb, btb, Wa, ba, Wc, bc)
    nc = _build_program()
    res = run_bass_kernel_spmd(
        nc, in_maps, core_ids=list(range(NCORES)), trace=trace
    )
    _last_results = res

    # headT[c] : [9, T*BL] with column index = ch*CW + tc*BL + b
    mean = np.empty((B, T, A), np.float32)
    value = np.empty((B, T), np.float32)
    for c in range(NCORES):
        r = res.results[c]["headT"].reshape(NHEAD, NCH, TC, BL)
        r = r.transpose(3, 1, 2, 0).reshape(BL, T, NHEAD)  # [b, t, 9]
        mean[c * BL:(c + 1) * BL] = r[:, :, :A]
        value[c * BL:(c + 1) * BL] = r[:, :, A]
    return mean, value


# revision 6
# speedup vs baseline: 2.2537x; 2.2537x over previous
"""CfCActorCritic forward_sequence kernel for Trainium2 (Bass/Tile), 8 cores.

Math (reference):
  per step t:  z  = 1.7159*tanh(0.666*([x_t, h] @ Wb + bb))
               ff1 = tanh(z @ Wff1 + bff1)
               ff2 = tanh(z @ Wff2 + bff2)
               ti  = sigmoid(z @ (Wta+Wtb) + bta+btb)        (ts = 1.0)
               h   = ff1*(1-ti) + ti*ff2
  heads: mean = out @ Wa + ba ; value = out @ Wc + bc  (per timestep)

Kernel reformulation (all algebraic, exact in fp32):
  - store state as h' = 2h; then with s = tanh((z@Wtab' + btab')),
    Wtab' = 0.5*1.7159*(Wta+Wtb), btab' = 0.5*(bta+btb):
      ti = (1+s)/2  and  h' = (1+s)*ff2 + (1-s)*ff1
    computed as p=(s+1)*ff2, q=(s-1)*ff1, h'=p-q  (2 STT + 1 TT on DVE)
  - 1.7159 folded into Wff1/Wff2/Wtab; 0.666 folded into Wb parts and bb;
    0.5 (from h'=2h) folded into Wb_h and the head weights.
  - obs projection x_t @ Wbx is batched: one N=512 matmul per 16-step chunk
    accumulated into a PSUM bank; the recurrent h@Wbh matmul accumulates
    into the same bank slice, so tanh_z reads PSUM directly (ACT bias=bb).
  - actor+critic heads are one [128,9] weight; applied once per chunk on the
    accumulated hidden-state tile [128, 512] (features x (t,b)).

Sharding: batch 256 -> 32 per core (data parallel, 8 cores); weights
replicated; scan over T=1024 sequential per core.
"""

import os
import sys
import types

import numpy as np

import concourse.bass as bass
import concourse.tile as tile
from concourse import mybir
from concourse.bass import ds

# ---------------------------------------------------------------- constants
B, T, OBS, H, A = 256, 1024, 32, 128, 8
NCORES = 8
BL = B // NCORES          # 32 batch per core
TC = 16                   # timesteps per chunk
NCH = T // TC             # 64 chunks
CW = TC * BL              # 512 chunk free width
NHEAD = A + 1             # actor(8) + critic(1)

_F32 = mybir.dt.float32

_last_results = None      # BassKernelResults from the most recent run


def _split_multi_waits(nc):
    """This toolchain's codegen rejects instructions carrying more than one
    sync-wait command. Hoist extra waits onto same-engine NOPs inserted just
    before the instruction (engine blocks on each in order — semantics
    identical, issue order per engine preserved)."""
    n_split = 0
    for f in nc.m.functions:
        for blk in f.blocks:
            new_insts = []
            for inst in blk.instructions:
                si = inst.sync_info
                if si is not None and len(si.on_wait) > 1:
                    waits = list(si.on_wait)
                    for k, w in enumerate(waits[:-1]):
                        nop = mybir.InstNoOp(
                            name=f"{inst.name}_sw{k}",
                            engine=inst.engine,
                            sync_info=mybir.SyncInfo(on_wait=[w], on_update=[]),
                        )
                        new_insts.append(nop)
                        n_split += 1
                    inst.sync_info = mybir.SyncInfo(
                        on_wait=[waits[-1]], on_update=list(si.on_update)
                    )
                new_insts.append(inst)
            blk.instructions[:] = new_insts
    return n_split


def _build_program():
    nc = bass.Bass()
    f32 = _F32

    bf = mybir.dt.bfloat16
    obs_r = nc.dram_tensor("obs_r", [NCH, OBS, CW], bf, kind="ExternalInput")
    wbx = nc.dram_tensor("wbx", [OBS, H], bf, kind="ExternalInput")
    wbh = nc.dram_tensor("wbh", [H, H], bf, kind="ExternalInput")
    wff1 = nc.dram_tensor("wff1", [H, H], bf, kind="ExternalInput")
    wff2 = nc.dram_tensor("wff2", [H, H], bf, kind="ExternalInput")
    wtab = nc.dram_tensor("wtab", [H, H], bf, kind="ExternalInput")
    wac = nc.dram_tensor("wac", [H, NHEAD], bf, kind="ExternalInput")
    bias3 = nc.dram_tensor("bias3", [3, H], bf, kind="ExternalInput")
    ind3 = nc.dram_tensor("ind3", [3, 3 * BL], bf, kind="ExternalInput")
    bbe = nc.dram_tensor("bbe", [H, 1], f32, kind="ExternalInput")
    bac = nc.dram_tensor("bac", [NHEAD, 1], f32, kind="ExternalInput")
    out = nc.dram_tensor("headT", [NHEAD, T * BL], f32, kind="ExternalOutput")

    Tanh = mybir.ActivationFunctionType.Tanh
    Identity = mybir.ActivationFunctionType.Identity
    ADD = mybir.AluOpType.add
    SUB = mybir.AluOpType.subtract
    MULT = mybir.AluOpType.mult

    with tile.TileContext(nc) as tc:
        with (
            tc.tile_pool(name="const", bufs=1) as const,
            tc.tile_pool(name="obsp", bufs=3) as obsp,
            tc.tile_pool(name="hchunk", bufs=2) as hchunkp,
            tc.tile_pool(name="outp", bufs=3) as outp,
            tc.tile_pool(name="zp", bufs=3) as zp,
            tc.tile_pool(name="fzp", bufs=3) as fzp,
            tc.tile_pool(name="tmpp", bufs=4) as tmpp,
            tc.tile_pool(name="xpp", bufs=2, space="PSUM") as xpp,
            tc.tile_pool(name="fpp", bufs=3, space="PSUM") as fpp,
            tc.tile_pool(name="hdp", bufs=2, space="PSUM") as hdp,
        ):
            # --- load constants into SBUF ---
            wbx_s = const.tile([OBS, H], bf)
            nc.sync.dma_start(out=wbx_s, in_=wbx[:])
            wbh_s = const.tile([H, H], bf)
            nc.sync.dma_start(out=wbh_s, in_=wbh[:])
            wff1_s = const.tile([H, H], bf)
            nc.sync.dma_start(out=wff1_s, in_=wff1[:])
            wff2_s = const.tile([H, H], bf)
            nc.sync.dma_start(out=wff2_s, in_=wff2[:])
            wtab_s = const.tile([H, H], bf)
            nc.sync.dma_start(out=wtab_s, in_=wtab[:])
            wac_s = const.tile([H, NHEAD], bf)
            nc.sync.dma_start(out=wac_s, in_=wac[:])
            bias3_s = const.tile([3, H], bf)
            nc.sync.dma_start(out=bias3_s, in_=bias3[:])
            ind3_s = const.tile([3, 3 * BL], bf)
            nc.sync.dma_start(out=ind3_s, in_=ind3[:])
            bbe_s = const.tile([H, 1], f32)
            nc.sync.dma_start(out=bbe_s, in_=bbe[:])
            bac_s = const.tile([NHEAD, 1], f32)
            nc.sync.dma_start(out=bac_s, in_=bac[:])

            prev_h = None
            for ch in range(NCH):
                obs_t = obsp.tile([OBS, CW], bf)
                nc.sync.dma_start(out=obs_t, in_=obs_r[ch])
                # xp[:, tc*32:(tc+1)*32] = 0.666 * x_t @ Wbx for the 16 steps
                xp = xpp.tile([H, CW], f32)
                nc.tensor.matmul(
                    xp, wbx_s, obs_t, start=True, stop=False, skip_group_check=True
                )
                hc = hchunkp.tile([H, CW], bf)
                for tcl in range(TC):
                    sl = ds(tcl * BL, BL)
                    if prev_h is not None:
                        nc.tensor.matmul(
                            xp[:, sl],
                            wbh_s,
                            prev_h,
                            start=False,
                            stop=True,
                            skip_group_check=True,
                        )
                    z = zp.tile([H, BL], bf)
                    nc.scalar.activation(z, xp[:, sl], Tanh, bias=bbe_s)

                    f_ps = fpp.tile([H, 3 * BL], f32)
                    nc.tensor.matmul(
                        f_ps, bias3_s, ind3_s, start=True, stop=False,
                        skip_group_check=True,
                    )
                    nc.tensor.matmul(
                        f_ps[:, ds(0, BL)], wff1_s, z, start=False, stop=False,
                        skip_group_check=True,
                    )
                    nc.tensor.matmul(
                        f_ps[:, ds(BL, BL)], wff2_s, z, start=False, stop=False,
                        skip_group_check=True,
                    )
                    nc.tensor.matmul(
                        f_ps[:, ds(2 * BL, BL)], wtab_s, z, start=False, stop=True,
                        skip_group_check=True,
                    )
                    fz = fzp.tile([H, 3 * BL], bf)
                    nc.scalar.activation(fz, f_ps, Tanh)
                    ff1 = fz[:, ds(0, BL)]
                    ff2 = fz[:, ds(BL, BL)]
                    s = fz[:, ds(2 * BL, BL)]
                    # h' = (1+s)*ff2 + (1-s)*ff1  ==  p - q
                    p = tmpp.tile([H, BL], bf)
                    nc.vector.scalar_tensor_tensor(p, s, 1.0, ff2, ADD, MULT)
                    q = tmpp.tile([H, BL], bf)
                    nc.vector.scalar_tensor_tensor(q, s, 1.0, ff1, SUB, MULT)
                    nc.vector.tensor_tensor(hc[:, sl], p, q, SUB)
                    prev_h = hc[:, sl]

                hd = hdp.tile([NHEAD, CW], f32)
                nc.tensor.matmul(hd, wac_s, hc, start=True, stop=True)
                ot = outp.tile([NHEAD, CW], f32)
                nc.scalar.activation(ot, hd, Identity, bias=bac_s)
                nc.sync.dma_start(out=out[:, ds(ch * CW, CW)], in_=ot)

    n = _split_multi_waits(nc)
    return nc


def _prep_inputs(obs, Wb, bb, Wff1, bff1, Wff2, bff2, Wta, bta, Wtb, btb,
                 Wa, ba, Wc, bc):
    """Fold scales into weights; reshape obs per-core. Returns list of in_maps."""
    import ml_dtypes

    bf16 = ml_dtypes.bfloat16
    a = 0.666
    g = 1.7159
    wbx = np.ascontiguousarray(a * Wb[:OBS, :]).astype(bf16)
    wbh = np.ascontiguousarray(0.5 * a * Wb[OBS:, :]).astype(bf16)
    bbe = (a * bb).astype(np.float32).reshape(H, 1)
    wff1 = np.ascontiguousarray(g * Wff1).astype(bf16)
    wff2 = np.ascontiguousarray(g * Wff2).astype(bf16)
    wtab = np.ascontiguousarray(0.5 * g * (Wta + Wtb)).astype(bf16)
    btab = 0.5 * (bta + btb)
    wac = np.ascontiguousarray(0.5 * np.concatenate([Wa, Wc], axis=1)).astype(
        bf16
    )  # [128, 9]
    bac = np.concatenate([ba, bc]).astype(np.float32).reshape(NHEAD, 1)
    bias3 = np.stack([bff1, bff2, btab]).astype(bf16)  # [3, 128]
    ind3 = np.zeros((3, 3 * BL), bf16)
    for gidx in range(3):
        ind3[gidx, gidx * BL:(gidx + 1) * BL] = 1

    # obs_r[c, ch, o, tc, b] = obs[c*BL + b, ch*TC + tc, o]
    obs_r = np.ascontiguousarray(
        obs.reshape(NCORES, BL, NCH, TC, OBS).transpose(0, 2, 4, 3, 1)
    ).astype(bf16).reshape(NCORES, NCH, OBS, CW)

    shared = dict(
        wbx=wbx, wbh=wbh, wff1=wff1, wff2=wff2, wtab=wtab, wac=wac,
        bbe=bbe, bias3=bias3, ind3=ind3, bac=bac,
    )
    return [dict(shared, obs_r=obs_r[c]) for c in range(NCORES)]


def _ensure_ntff_hook():
    """Register the axon NTFF profiling hook if the image's antenv lacks it."""
    try:
        from antenv.axon_hooks import get_axon_ntff_profile_hook  # noqa: F401
        return
    except ImportError:
        pass
    import antenv

    mod = types.ModuleType("antenv.axon_hooks")
    _h = [None]
    mod.set_axon_ntff_profile_hook = lambda hook: _h.__setitem__(0, hook)
    mod.get_axon_ntff_profile_hook = lambda: _h[0]
    sys.modules["antenv.axon_hooks"] = mod
    antenv.axon_hooks = mod
    try:
        from trn_agent_boot.trn_boot import _ntff_profile_via_ctypes

        mod.set_axon_ntff_profile_hook(
            _ntff_profile_via_ctypes("/opt/axon/libaxon_pjrt.so")
        )
    except Exception:
        pass


def kernel(obs, Wb, bb, Wff1, bff1, Wff2, bff2, Wta, bta, Wtb, btb,
           Wa, ba, Wc, bc):
    global _last_results
    from concourse.bass_utils import run_bass_kernel_spmd

    trace = os.environ.get("CFC_TRACE", "") == "1"
    if trace:
        _ensure_ntff_hook()

    in_maps = _prep_inputs(obs, Wb, bb, Wff1, bff1, Wff2, bff2, Wta, bta,
                           Wtb, btb, Wa, ba, Wc, bc)
    nc = _build_program()
    res = run_bass_kernel_spmd(
        nc, in_maps, core_ids=list(range(NCORES)), trace=trace
    )
    _last_results = res

    # headT[c] : [9, T*BL] with column index = ch*CW + tc*BL + b
    mean = np.empty((B, T, A), np.float32)
    value = np.empty((B, T), np.float32)
    for c in range(NCORES):
        r = res.results[c]["headT"].reshape(NHEAD, NCH, TC, BL)
        r = r.transpose(3, 1, 2, 0).reshape(BL, T, NHEAD)  # [b, t, 9]
        mean[c * BL:(c + 1) * BL] = r[:, :, :A]
        value[c * BL:(c + 1) * BL] = r[:, :, A]
    return mean, value
